# revision 37
# baseline (speedup 1.0000x reference)
"""HashLayerFFN Trainium2 kernel — H-split expert pairs.

Routing model: each token picks one of E=8 expert FFNs via a hash map; the
host groups tokens by expert (cheap numpy argsort).  Naive expert-parallel
(one expert per core) is PE-bound at the LARGEST bucket (310 of 2048
tokens, 21% over the mean), so instead experts are PAIRED (largest bucket
with smallest) and each pair maps to TWO cores: a core holds HALF the
hidden dim (1024 of 2048 rows) of BOTH experts in its pair and processes
ALL their tokens.  The two cores' partial y are summed on the host during
the unshard (b2 added there too — partial sums make on-device bias wrong).
Per-core PE work is thereby balanced at (capA+capB)=562 token-columns
instead of 620 for capacity-padded expert-parallel, and per-core weight
traffic stays at exactly one expert-equivalent (8MB fp16).

Device schedule per core — warmup, 1A, 2A, 1B, 2B, all chasing ONE packed
DMA stream [A-segs (x_A-d | W1A-half-d) x8, W2A h-chunks x8, B-segs x8,
W2B x8]:
 - Packing the stream on the host keeps every DMA >=2KB/partition, so
   serialized descriptor generation (HWDGE ~625ns/DMA) never starves the
   wire, and segment order matches consumption order exactly.  seg1 rides
   the second HWDGE queue: a single queue's ~650ns issue cadence would
   otherwise delay it behind seg0's descriptor chain and stall the chase.
 - Phase 1A runs its 8 h-tiles d-interleaved across all 8 PSUM banks:
   consumption (~1034ns/seg) just exceeds supply (~949ns/seg), so the PE
   runs gap-free from the first segment's arrival (~3.8us) to the last
   matmul.  The last two d-sweeps of each phase-1 close h=0/1 early so
   the relu chain overlaps the remaining matmuls and phase 2 starts
   without a boundary stall.
 - The PE p-state ramp (0.65/1.2GHz until ~3us of continuous matmul) is
   burned off beforehand by warm-up matmuls on a zeroed tile.
 - Phases 2A/2B accumulate h-major while chasing the W2 chunks, then
   switch d-major so PSUM groups close staggered and the fp16 y stores
   (batched 4-wide to amortize HWDGE, B-width padded to a 512B element)
   overlap the remaining matmuls.

PE matmuls are fp16 (10-bit mantissa, ~5e-04 max relative error vs the
fp32 reference) with fp32 PSUM accumulation; fp16 halves the weight
stream vs fp32.  fp8/DoubleRow was measured numerically at 2.3-5.3e-02
error (over the 2e-02 gate) unless hi-lo compensated, which costs 3x the
matmuls — rejected.
"""

import numpy as np

B, S, D, H, E = 2, 1024, 1024, 2048, 8
N_CORES = 8
NP = 4             # expert pairs
CA = 310           # capacity for the big expert of each pair
CB = 252           # capacity for the small expert
DT8 = 8            # h-tiles per half (H/2 = 1024)
ND = 8             # d chunks of 128 in D

N_WARM = 112
WARM_COLS = 32

SEG_A = CA + 1024
SEG_B = CB + 1024
OFF_W2A = ND * SEG_A
OFF_B = OFF_W2A + DT8 * 1024
OFF_W2B = OFF_B + ND * SEG_B
TOT = OFF_W2B + DT8 * 1024

MODE = "fp16_fp16"   # kept for test.py compatibility
RUN_KWARGS = {}
LAST_RES = None
_cache = {}


def _build_nc():
    import concourse.mybir as mybir
    from concourse import bacc
    from concourse.tile import TileContext

    f32 = mybir.dt.float32
    f16 = mybir.dt.float16
    dt1 = f16

    nc = bacc.Bacc(None, target_bir_lowering=False)
    st = nc.dram_tensor("st", [128, TOT], dt1, kind="ExternalInput")
    bt = nc.dram_tensor("bt", [128, 2 * DT8], f32, kind="ExternalInput")
    yta = nc.dram_tensor("yta", [128, ND, CA], f16, kind="ExternalOutput")
    CBP = 256          # B store width padded to a 512B elem
    ytb = nc.dram_tensor("ytb", [128, ND, CBP], f16, kind="ExternalOutput")

    with TileContext(nc) as tc:
        with (
            tc.tile_pool(name="consts", bufs=1) as consts,
            tc.tile_pool(name="spool", bufs=1) as spool,
            tc.tile_pool(name="hpool", bufs=1) as hpool,
            tc.tile_pool(name="yapool", bufs=1) as yapool,
            tc.tile_pool(name="ybpool", bufs=1) as ybpool,
            tc.tile_pool(name="warm", bufs=1) as warmp,
            tc.tile_pool(name="ps1p", bufs=4, space="PSUM") as ps1p,
            tc.tile_pool(name="ps2p", bufs=4, space="PSUM") as ps2p,
        ):
            wtile = warmp.tile([128, 128], dt1, name="wtile")
            nc.vector.memset(wtile, 0)

            stile = spool.tile([128, TOT], dt1, name="stile")
            # A-segs alternate between the two HWDGE queues: a single queue's
            # ~650ns issue cadence would delay seg1 behind seg0's descriptor
            # chain; alternating lets transfers pack back-to-back on the wire
            for d in range(ND):
                q = nc.scalar if d == 1 else nc.sync
                q.dma_start(
                    out=stile[:, d * SEG_A:(d + 1) * SEG_A],
                    in_=st[:, d * SEG_A:(d + 1) * SEG_A],
                )
            bts = consts.tile([128, 2 * DT8], f32)
            b1a, b1b = bts[:, 0:DT8], bts[:, DT8:2 * DT8]
            nc.scalar.dma_start(out=bts, in_=bt[:])
            for j in range(DT8):
                nc.sync.dma_start(
                    out=stile[:, OFF_W2A + j * 1024:OFF_W2A + (j + 1) * 1024],
                    in_=st[:, OFF_W2A + j * 1024:OFF_W2A + (j + 1) * 1024],
                )
            for d in range(ND):
                nc.sync.dma_start(
                    out=stile[:, OFF_B + d * SEG_B:OFF_B + (d + 1) * SEG_B],
                    in_=st[:, OFF_B + d * SEG_B:OFF_B + (d + 1) * SEG_B],
                )
            for j in range(DT8):
                nc.sync.dma_start(
                    out=stile[:, OFF_W2B + j * 1024:OFF_W2B + (j + 1) * 1024],
                    in_=st[:, OFF_W2B + j * 1024:OFF_W2B + (j + 1) * 1024],
                )

            xa = [stile[:, d * SEG_A:d * SEG_A + CA] for d in range(ND)]
            xb = [stile[:, OFF_B + d * SEG_B:OFF_B + d * SEG_B + CB]
                  for d in range(ND)]

            def w1a(d, j):
                off = d * SEG_A + CA + j * 128
                return stile[:, off:off + 128]

            def w1b(d, j):
                off = OFF_B + d * SEG_B + CB + j * 128
                return stile[:, off:off + 128]

            def w2a(j, d):
                off = OFF_W2A + j * 1024 + d * 128
                return stile[:, off:off + 128]

            def w2b(j, d):
                off = OFF_W2B + j * 1024 + d * 128
                return stile[:, off:off + 128]

            # ---- phase 1A: 8 h-tiles d-interleaved, chasing A-segs
            psa = [ps1p.tile([128, CA], f32, name="ps1") for _ in range(4)] + \
                  [ps2p.tile([128, CA], f32, name="ps2") for _ in range(4)]
            for i in range(N_WARM):
                pw = psa[1 + i % 7]
                nc.tensor.matmul(
                    pw[:, :WARM_COLS], lhsT=wtile, rhs=wtile[:, :WARM_COLS],
                    start=True, stop=True,
                )
            # last two d-sweeps reordered so psa[0] (and [1]) close early:
            # their relus then overlap the remaining matmuls and phase 2A
            # starts without waiting on the h=0 relu chain.
            def phase1(ps, w1, xs):
                for d in range(ND - 2):
                    for j in range(DT8):
                        nc.tensor.matmul(
                            ps[j], lhsT=w1(d, j), rhs=xs[d],
                            start=(d == 0), stop=False,
                        )
                d6, d7 = ND - 2, ND - 1
                for j in range(4):
                    nc.tensor.matmul(
                        ps[j], lhsT=w1(d6, j), rhs=xs[d6],
                        start=False, stop=False,
                    )
                for j in range(2):
                    nc.tensor.matmul(
                        ps[j], lhsT=w1(d7, j), rhs=xs[d7],
                        start=False, stop=True,
                    )
                for j in range(4, DT8):
                    nc.tensor.matmul(
                        ps[j], lhsT=w1(d6, j), rhs=xs[d6],
                        start=False, stop=False,
                    )
                for j in range(2, DT8):
                    nc.tensor.matmul(
                        ps[j], lhsT=w1(d7, j), rhs=xs[d7],
                        start=False, stop=True,
                    )

            phase1(psa, w1a, xa)
            hida = []
            for j in range(DT8):
                hid = hpool.tile([128, CA], dt1, name=f"hida{j}")
                nc.scalar.activation(
                    out=hid, in_=psa[j],
                    func=mybir.ActivationFunctionType.Relu,
                    bias=b1a[:, j:j + 1],
                )
                hida.append(hid)

            # ---- phase 2A: 8 d-groups, j-major while chasing W2A, then
            # d-major so groups close staggered
            pga = [ps2p.tile([128, CA], f32, name="ps2") for _ in range(4)] + \
                  [ps1p.tile([128, CA], f32, name="ps1") for _ in range(4)]
            J1 = DT8 // 2
            for j in range(J1):
                for d in range(ND):
                    nc.tensor.matmul(
                        pga[d], lhsT=w2a(j, d), rhs=hida[j],
                        start=(j == 0), stop=False,
                    )
            ya = yapool.tile([128, ND, CA], f16, name="ya")
            for d in range(ND):
                for j in range(J1, DT8):
                    nc.tensor.matmul(
                        pga[d], lhsT=w2a(j, d), rhs=hida[j],
                        start=False, stop=(j == DT8 - 1),
                    )
                if d % 2 == 0:
                    nc.scalar.activation(
                        out=ya[:, d, :], in_=pga[d],
                        func=mybir.ActivationFunctionType.Identity,
                    )
                else:
                    nc.vector.tensor_copy(ya[:, d, :], pga[d])
                if d == 3:
                    nc.sync.dma_start(out=yta[:, 0:4, :], in_=ya[:, 0:4, :])
                elif d == ND - 1:
                    nc.sync.dma_start(out=yta[:, 4:8, :], in_=ya[:, 4:8, :])

            # ---- phase 1B
            psb = [ps1p.tile([128, CB], f32, name="ps1") for _ in range(4)] + \
                  [ps2p.tile([128, CB], f32, name="ps2") for _ in range(4)]
            phase1(psb, w1b, xb)
            hidb = []
            for j in range(DT8):
                hid = hpool.tile([128, CB], dt1, name=f"hidb{j}")
                nc.scalar.activation(
                    out=hid, in_=psb[j],
                    func=mybir.ActivationFunctionType.Relu,
                    bias=b1b[:, j:j + 1],
                )
                hidb.append(hid)

            # ---- phase 2B
            pgb = [ps2p.tile([128, CB], f32, name="ps2") for _ in range(4)] + \
                  [ps1p.tile([128, CB], f32, name="ps1") for _ in range(4)]
            for j in range(J1):
                for d in range(ND):
                    nc.tensor.matmul(
                        pgb[d], lhsT=w2b(j, d), rhs=hidb[j],
                        start=(j == 0), stop=False,
                    )
            yb = ybpool.tile([128, ND, CBP], f16, name="yb")
            nc.vector.memset(yb[:, :, CB:CBP], 0)
            for d in range(ND):
                for j in range(J1, DT8):
                    nc.tensor.matmul(
                        pgb[d], lhsT=w2b(j, d), rhs=hidb[j],
                        start=False, stop=(j == DT8 - 1),
                    )
                if d % 2 == 0:
                    nc.scalar.activation(
                        out=yb[:, d, 0:CB], in_=pgb[d],
                        func=mybir.ActivationFunctionType.Identity,
                    )
                else:
                    nc.vector.tensor_copy(yb[:, d, 0:CB], pgb[d])
                if d == 3:
                    nc.sync.dma_start(out=ytb[:, 0:4, :], in_=yb[:, 0:4, :])
                elif d == 6:
                    nc.scalar.dma_start(out=ytb[:, 4:7, :], in_=yb[:, 4:7, :])
                elif d == ND - 1:
                    nc.sync.dma_start(out=ytb[:, 7:8, :], in_=yb[:, 7:8, :])

    nc.finalize()
    return nc


def get_nc():
    if "b" not in _cache:
        _cache["b"] = _build_nc()
    return _cache["b"]


_get_nc = get_nc   # test.py compatibility


def kernel(x, orig_input, hash_map, W1, b1, W2, b2, **_unused):
    from concourse import bass_utils

    x = np.asarray(x)
    W1 = np.asarray(W1, dtype=np.float32)
    b1 = np.asarray(b1, dtype=np.float32)
    W2 = np.asarray(W2, dtype=np.float32)
    b2 = np.asarray(b2, dtype=np.float32)

    xf = np.ascontiguousarray(x, dtype=np.float32).reshape(B * S, D)
    e = np.asarray(hash_map).astype(np.int64)[
        np.asarray(orig_input).astype(np.int64).reshape(-1)
    ]
    order = np.argsort(e, kind="stable")
    counts = np.bincount(e, minlength=E)
    starts = np.zeros(E + 1, dtype=np.int64)
    starts[1:] = np.cumsum(counts)

    # pair largest with smallest
    rank = np.argsort(-counts, kind="stable")
    pairs = [(int(rank[p]), int(rank[E - 1 - p])) for p in range(NP)]

    overflow = []
    tok = {}
    for i in range(E):
        cap = CA if any(p[0] == i for p in pairs) else CB
        idx = order[starts[i]:starts[i + 1]]
        if len(idx) > cap:
            overflow.append((i, idx[cap:]))
            idx = idx[:cap]
        tok[i] = idx

    def xpack(idx, cap):
        xe = np.zeros((cap, D), dtype=np.float32)
        xe[: len(idx)] = xf[idx]
        return xe.T.reshape(ND, 128, cap)       # [d, 128, cap]

    in_maps = []
    for p, (a, b) in enumerate(pairs):
        xta = xpack(tok[a], CA).astype(np.float16)
        xtb = xpack(tok[b], CB).astype(np.float16)
        for hh in range(2):
            r0 = hh * 1024
            w1ah = W1[a][r0:r0 + 1024].T.reshape(ND, 128, DT8, 128)
            w1bh = W1[b][r0:r0 + 1024].T.reshape(ND, 128, DT8, 128)
            w2ah = W2[a][:, r0:r0 + 1024].T.reshape(DT8, 128, D)
            w2bh = W2[b][:, r0:r0 + 1024].T.reshape(DT8, 128, D)
            stream = np.empty((128, TOT), dtype=np.float16)
            for d in range(ND):
                seg = stream[:, d * SEG_A:(d + 1) * SEG_A]
                seg[:, :CA] = xta[d]
                seg[:, CA:] = w1ah[d].reshape(128, 1024)
                seg = stream[:, OFF_B + d * SEG_B:OFF_B + (d + 1) * SEG_B]
                seg[:, :CB] = xtb[d]
                seg[:, CB:] = w1bh[d].reshape(128, 1024)
            stream[:, OFF_W2A:OFF_W2A + DT8 * 1024] = \
                w2ah.transpose(1, 0, 2).reshape(128, DT8 * 1024)
            stream[:, OFF_W2B:] = \
                w2bh.transpose(1, 0, 2).reshape(128, DT8 * 1024)
            bta = b1[a][r0:r0 + 1024].reshape(DT8, 128).T
            btb = b1[b][r0:r0 + 1024].reshape(DT8, 128).T
            in_maps.append({
                "st": stream,
                "bt": np.ascontiguousarray(
                    np.concatenate([bta, btb], axis=1)),
            })

    nc = get_nc()
    try:
        res = bass_utils.run_bass_kernel_spmd(
            nc, in_maps, core_ids=list(range(N_CORES)), **RUN_KWARGS
        )
    except Exception:
        # The axon-tunneled devices intermittently fail with
        # NRT_EXEC_UNIT_UNRECOVERABLE (~5% of launches, kernel-independent);
        # a single retry has always recovered.
        import time
        time.sleep(2.0)
        res = bass_utils.run_bass_kernel_spmd(
            nc, in_maps, core_ids=list(range(N_CORES)), **RUN_KWARGS
        )
    global LAST_RES
    LAST_RES = res

    out = np.zeros((B * S, D), dtype=np.float32)
    for p, (a, b) in enumerate(pairs):
        r0, r1 = res.results[2 * p], res.results[2 * p + 1]
        ya = (r0["yta"].astype(np.float32) + r1["yta"].astype(np.float32))
        yb = (r0["ytb"].astype(np.float32) + r1["ytb"].astype(np.float32))
        ya = ya.transpose(1, 0, 2).reshape(D, CA).T + b2[a]   # [CA, D]
        yb = yb[:, :, :CB].transpose(1, 0, 2).reshape(D, CB).T + b2[b]
        out[tok[a]] = ya[: len(tok[a])]
        out[tok[b]] = yb[: len(tok[b])]
    for i, idx in overflow:
        hh = np.maximum(xf[idx] @ W1[i].T + b1[i], 0.0)
        out[idx] = hh @ W2[i].T + b2[i]
    return out.reshape(B, S, D)


# revision 41
# speedup vs baseline: 1.0005x; 1.0005x over previous
"""HashLayerFFN Trainium2 kernel — H-split expert pairs.

Routing model: each token picks one of E=8 expert FFNs via a hash map; the
host groups tokens by expert (cheap numpy argsort).  Naive expert-parallel
(one expert per core) is PE-bound at the LARGEST bucket (310 of 2048
tokens, 21% over the mean), so instead experts are PAIRED (largest bucket
with smallest) and each pair maps to TWO cores: a core holds HALF the
hidden dim (1024 of 2048 rows) of BOTH experts in its pair and processes
ALL their tokens.  The two cores' partial y are summed on the host during
the unshard (b2 added there too — partial sums make on-device bias wrong).
Per-core PE work is thereby balanced at (capA+capB)=562 token-columns
instead of 620 for capacity-padded expert-parallel, and per-core weight
traffic stays at exactly one expert-equivalent (8MB fp16).

Device schedule per core — warmup, 1A, 2A, 1B, 2B, all chasing ONE packed
DMA stream [A-segs (x_A-d | W1A-half-d) x8, W2A h-chunks x8, B-segs x8,
W2B x8]:
 - Packing the stream on the host keeps every DMA >=2KB/partition, so
   serialized descriptor generation (HWDGE ~625ns/DMA) never starves the
   wire, and segment order matches consumption order exactly.  seg1 rides
   the second HWDGE queue: a single queue's ~650ns issue cadence would
   otherwise delay it behind seg0's descriptor chain and stall the chase.
 - Phase 1A runs its 8 h-tiles d-interleaved across all 8 PSUM banks:
   consumption (~1034ns/seg) just exceeds supply (~949ns/seg), so the PE
   runs gap-free from the first segment's arrival (~3.8us) to the last
   matmul.  The last two d-sweeps of each phase-1 close h=0/1 early so
   the relu chain overlaps the remaining matmuls and phase 2 starts
   without a boundary stall.
 - The PE p-state ramp (0.65/1.2GHz until ~3us of continuous matmul) is
   burned off beforehand by warm-up matmuls on a zeroed tile.
 - Phases 2A/2B accumulate h-major while chasing the W2 chunks, then
   switch d-major so PSUM groups close staggered and the fp16 y stores
   (batched 4-wide to amortize HWDGE, B-width padded to a 512B element)
   overlap the remaining matmuls.

PE matmuls are fp16 (10-bit mantissa, ~5e-04 max relative error vs the
fp32 reference) with fp32 PSUM accumulation; fp16 halves the weight
stream vs fp32.  fp8/DoubleRow was measured numerically at 2.3-5.3e-02
error (over the 2e-02 gate) unless hi-lo compensated, which costs 3x the
matmuls — rejected.
"""

import numpy as np

B, S, D, H, E = 2, 1024, 1024, 2048, 8
N_CORES = 8
NP = 4             # expert pairs
CA = 310           # capacity for the big expert of each pair
CB = 252           # capacity for the small expert
DT8 = 8            # h-tiles per half (H/2 = 1024)
ND = 8             # d chunks of 128 in D

N_WARM = 112
WARM_COLS = 32

SEG_A = CA + 1024
SEG_B = CB + 1024
OFF_W2A = ND * SEG_A
OFF_B = OFF_W2A + DT8 * 1024
OFF_W2B = OFF_B + ND * SEG_B
TOT = OFF_W2B + DT8 * 1024

MODE = "fp16_fp16"   # kept for test.py compatibility
RUN_KWARGS = {}
LAST_RES = None
_cache = {}


def _build_nc():
    import concourse.mybir as mybir
    from concourse import bacc
    from concourse.tile import TileContext

    f32 = mybir.dt.float32
    f16 = mybir.dt.float16
    dt1 = f16

    nc = bacc.Bacc(None, target_bir_lowering=False)
    st = nc.dram_tensor("st", [128, TOT], dt1, kind="ExternalInput")
    bt = nc.dram_tensor("bt", [128, 2 * DT8], f32, kind="ExternalInput")
    yta = nc.dram_tensor("yta", [128, ND, CA], f16, kind="ExternalOutput")
    CBP = 256          # B store width padded to a 512B elem
    ytb = nc.dram_tensor("ytb", [128, ND, CBP], f16, kind="ExternalOutput")

    with TileContext(nc) as tc:
        with (
            tc.tile_pool(name="consts", bufs=1) as consts,
            tc.tile_pool(name="spool", bufs=1) as spool,
            tc.tile_pool(name="hpool", bufs=1) as hpool,
            tc.tile_pool(name="yapool", bufs=1) as yapool,
            tc.tile_pool(name="ybpool", bufs=1) as ybpool,
            tc.tile_pool(name="warm", bufs=1) as warmp,
            tc.tile_pool(name="ps1p", bufs=4, space="PSUM") as ps1p,
            tc.tile_pool(name="ps2p", bufs=4, space="PSUM") as ps2p,
        ):
            wtile = warmp.tile([128, 128], dt1, name="wtile")
            nc.vector.memset(wtile, 0)

            stile = spool.tile([128, TOT], dt1, name="stile")
            # A-segs alternate between the two HWDGE queues: a single queue's
            # ~650ns issue cadence would delay seg1 behind seg0's descriptor
            # chain; alternating lets transfers pack back-to-back on the wire
            for d in range(ND):
                q = nc.scalar if d == 1 else nc.sync
                q.dma_start(
                    out=stile[:, d * SEG_A:(d + 1) * SEG_A],
                    in_=st[:, d * SEG_A:(d + 1) * SEG_A],
                )
            bts = consts.tile([128, 2 * DT8], f32)
            b1a, b1b = bts[:, 0:DT8], bts[:, DT8:2 * DT8]
            nc.scalar.dma_start(out=bts, in_=bt[:])
            for j in range(DT8):
                nc.sync.dma_start(
                    out=stile[:, OFF_W2A + j * 1024:OFF_W2A + (j + 1) * 1024],
                    in_=st[:, OFF_W2A + j * 1024:OFF_W2A + (j + 1) * 1024],
                )
            for d in range(ND):
                nc.sync.dma_start(
                    out=stile[:, OFF_B + d * SEG_B:OFF_B + (d + 1) * SEG_B],
                    in_=st[:, OFF_B + d * SEG_B:OFF_B + (d + 1) * SEG_B],
                )
            for j in range(DT8):
                nc.sync.dma_start(
                    out=stile[:, OFF_W2B + j * 1024:OFF_W2B + (j + 1) * 1024],
                    in_=st[:, OFF_W2B + j * 1024:OFF_W2B + (j + 1) * 1024],
                )

            xa = [stile[:, d * SEG_A:d * SEG_A + CA] for d in range(ND)]
            xb = [stile[:, OFF_B + d * SEG_B:OFF_B + d * SEG_B + CB]
                  for d in range(ND)]

            def w1a(d, j):
                off = d * SEG_A + CA + j * 128
                return stile[:, off:off + 128]

            def w1b(d, j):
                off = OFF_B + d * SEG_B + CB + j * 128
                return stile[:, off:off + 128]

            def w2a(j, d):
                off = OFF_W2A + j * 1024 + d * 128
                return stile[:, off:off + 128]

            def w2b(j, d):
                off = OFF_W2B + j * 1024 + d * 128
                return stile[:, off:off + 128]

            # ---- phase 1A: 8 h-tiles d-interleaved, chasing A-segs
            psa = [ps1p.tile([128, CA], f32, name="ps1") for _ in range(4)] + \
                  [ps2p.tile([128, CA], f32, name="ps2") for _ in range(4)]
            for i in range(N_WARM):
                pw = psa[1 + i % 7]
                nc.tensor.matmul(
                    pw[:, :WARM_COLS], lhsT=wtile, rhs=wtile[:, :WARM_COLS],
                    start=True, stop=True,
                )
            # last two d-sweeps reordered so psa[0] (and [1]) close early:
            # their relus then overlap the remaining matmuls and phase 2A
            # starts without waiting on the h=0 relu chain.
            def phase1(ps, w1, xs):
                for d in range(ND - 2):
                    for j in range(DT8):
                        nc.tensor.matmul(
                            ps[j], lhsT=w1(d, j), rhs=xs[d],
                            start=(d == 0), stop=False,
                        )
                d6, d7 = ND - 2, ND - 1
                for j in range(4):
                    nc.tensor.matmul(
                        ps[j], lhsT=w1(d6, j), rhs=xs[d6],
                        start=False, stop=False,
                    )
                for j in range(2):
                    nc.tensor.matmul(
                        ps[j], lhsT=w1(d7, j), rhs=xs[d7],
                        start=False, stop=True,
                    )
                for j in range(4, DT8):
                    nc.tensor.matmul(
                        ps[j], lhsT=w1(d6, j), rhs=xs[d6],
                        start=False, stop=False,
                    )
                for j in range(2, DT8):
                    nc.tensor.matmul(
                        ps[j], lhsT=w1(d7, j), rhs=xs[d7],
                        start=False, stop=True,
                    )

            phase1(psa, w1a, xa)
            hida = []
            for j in range(DT8):
                hid = hpool.tile([128, CA], dt1, name=f"hida{j}")
                nc.scalar.activation(
                    out=hid, in_=psa[j],
                    func=mybir.ActivationFunctionType.Relu,
                    bias=b1a[:, j:j + 1],
                )
                hida.append(hid)

            # ---- phase 2A: 8 d-groups, j-major while chasing W2A, then
            # d-major so groups close staggered
            pga = [ps2p.tile([128, CA], f32, name="ps2") for _ in range(4)] + \
                  [ps1p.tile([128, CA], f32, name="ps1") for _ in range(4)]
            J1 = DT8 // 2
            for j in range(J1):
                for d in range(ND):
                    nc.tensor.matmul(
                        pga[d], lhsT=w2a(j, d), rhs=hida[j],
                        start=(j == 0), stop=False,
                    )
            ya = yapool.tile([128, ND, CA], f16, name="ya")
            for d in range(ND - 1, -1, -1):
                for j in range(J1, DT8):
                    nc.tensor.matmul(
                        pga[d], lhsT=w2a(j, d), rhs=hida[j],
                        start=False, stop=(j == DT8 - 1),
                    )
                if d % 2 == 0:
                    nc.scalar.activation(
                        out=ya[:, d, :], in_=pga[d],
                        func=mybir.ActivationFunctionType.Identity,
                    )
                else:
                    nc.vector.tensor_copy(ya[:, d, :], pga[d])
                if d == 4:
                    nc.sync.dma_start(out=yta[:, 4:8, :], in_=ya[:, 4:8, :])
                elif d == 0:
                    nc.sync.dma_start(out=yta[:, 0:4, :], in_=ya[:, 0:4, :])

            # ---- phase 1B
            psb = [ps1p.tile([128, CB], f32, name="ps1") for _ in range(4)] + \
                  [ps2p.tile([128, CB], f32, name="ps2") for _ in range(4)]
            phase1(psb, w1b, xb)
            hidb = []
            for j in range(DT8):
                hid = hpool.tile([128, CB], dt1, name=f"hidb{j}")
                nc.scalar.activation(
                    out=hid, in_=psb[j],
                    func=mybir.ActivationFunctionType.Relu,
                    bias=b1b[:, j:j + 1],
                )
                hidb.append(hid)

            # ---- phase 2B
            pgb = [ps2p.tile([128, CB], f32, name="ps2") for _ in range(4)] + \
                  [ps1p.tile([128, CB], f32, name="ps1") for _ in range(4)]
            for j in range(J1):
                for d in range(ND):
                    nc.tensor.matmul(
                        pgb[d], lhsT=w2b(j, d), rhs=hidb[j],
                        start=(j == 0), stop=False,
                    )
            yb = ybpool.tile([128, ND, CBP], f16, name="yb")
            nc.vector.memset(yb[:, :, CB:CBP], 0)
            for d in range(ND - 1, -1, -1):
                for j in range(J1, DT8):
                    nc.tensor.matmul(
                        pgb[d], lhsT=w2b(j, d), rhs=hidb[j],
                        start=False, stop=(j == DT8 - 1),
                    )
                if d % 2 == 0:
                    nc.scalar.activation(
                        out=yb[:, d, 0:CB], in_=pgb[d],
                        func=mybir.ActivationFunctionType.Identity,
                    )
                else:
                    nc.vector.tensor_copy(yb[:, d, 0:CB], pgb[d])
                if d == 4:
                    nc.sync.dma_start(out=ytb[:, 4:8, :], in_=yb[:, 4:8, :])
                elif d == 1:
                    nc.scalar.dma_start(out=ytb[:, 1:4, :], in_=yb[:, 1:4, :])
                elif d == 0:
                    nc.sync.dma_start(out=ytb[:, 0:1, :], in_=yb[:, 0:1, :])

    nc.finalize()
    return nc


def get_nc():
    if "b" not in _cache:
        _cache["b"] = _build_nc()
    return _cache["b"]


_get_nc = get_nc   # test.py compatibility


def kernel(x, orig_input, hash_map, W1, b1, W2, b2, **_unused):
    from concourse import bass_utils

    x = np.asarray(x)
    W1 = np.asarray(W1, dtype=np.float32)
    b1 = np.asarray(b1, dtype=np.float32)
    W2 = np.asarray(W2, dtype=np.float32)
    b2 = np.asarray(b2, dtype=np.float32)

    xf = np.ascontiguousarray(x, dtype=np.float32).reshape(B * S, D)
    e = np.asarray(hash_map).astype(np.int64)[
        np.asarray(orig_input).astype(np.int64).reshape(-1)
    ]
    order = np.argsort(e, kind="stable")
    counts = np.bincount(e, minlength=E)
    starts = np.zeros(E + 1, dtype=np.int64)
    starts[1:] = np.cumsum(counts)

    # pair largest with smallest
    rank = np.argsort(-counts, kind="stable")
    pairs = [(int(rank[p]), int(rank[E - 1 - p])) for p in range(NP)]

    overflow = []
    tok = {}
    for i in range(E):
        cap = CA if any(p[0] == i for p in pairs) else CB
        idx = order[starts[i]:starts[i + 1]]
        if len(idx) > cap:
            overflow.append((i, idx[cap:]))
            idx = idx[:cap]
        tok[i] = idx

    def xpack(idx, cap):
        xe = np.zeros((cap, D), dtype=np.float32)
        xe[: len(idx)] = xf[idx]
        return xe.T.reshape(ND, 128, cap)       # [d, 128, cap]

    in_maps = []
    for p, (a, b) in enumerate(pairs):
        xta = xpack(tok[a], CA).astype(np.float16)
        xtb = xpack(tok[b], CB).astype(np.float16)
        for hh in range(2):
            r0 = hh * 1024
            w1ah = W1[a][r0:r0 + 1024].T.reshape(ND, 128, DT8, 128)
            w1bh = W1[b][r0:r0 + 1024].T.reshape(ND, 128, DT8, 128)
            w2ah = W2[a][:, r0:r0 + 1024].T.reshape(DT8, 128, D)
            w2bh = W2[b][:, r0:r0 + 1024].T.reshape(DT8, 128, D)
            stream = np.empty((128, TOT), dtype=np.float16)
            for d in range(ND):
                seg = stream[:, d * SEG_A:(d + 1) * SEG_A]
                seg[:, :CA] = xta[d]
                seg[:, CA:] = w1ah[d].reshape(128, 1024)
                seg = stream[:, OFF_B + d * SEG_B:OFF_B + (d + 1) * SEG_B]
                seg[:, :CB] = xtb[d]
                seg[:, CB:] = w1bh[d].reshape(128, 1024)
            stream[:, OFF_W2A:OFF_W2A + DT8 * 1024] = \
                w2ah.transpose(1, 0, 2).reshape(128, DT8 * 1024)
            stream[:, OFF_W2B:] = \
                w2bh.transpose(1, 0, 2).reshape(128, DT8 * 1024)
            bta = b1[a][r0:r0 + 1024].reshape(DT8, 128).T
            btb = b1[b][r0:r0 + 1024].reshape(DT8, 128).T
            in_maps.append({
                "st": stream,
                "bt": np.ascontiguousarray(
                    np.concatenate([bta, btb], axis=1)),
            })

    nc = get_nc()
    try:
        res = bass_utils.run_bass_kernel_spmd(
            nc, in_maps, core_ids=list(range(N_CORES)), **RUN_KWARGS
        )
    except Exception:
        # The axon-tunneled devices intermittently fail with
        # NRT_EXEC_UNIT_UNRECOVERABLE (~5% of launches, kernel-independent;
        # the same NEFF passes on re-run).  Retry once in-process; if the
        # device stays wedged, fall back to a fresh process, which has
        # always recovered.
        import os
        import time
        if os.environ.get("_HLFFN_SUBPROC") == "1":
            raise
        try:
            time.sleep(2.0)
            res = bass_utils.run_bass_kernel_spmd(
                nc, in_maps, core_ids=list(range(N_CORES)), **RUN_KWARGS
            )
        except Exception:
            return _fresh_process_fallback(
                x, orig_input, hash_map, W1, b1, W2, b2)
    global LAST_RES
    LAST_RES = res

    out = np.zeros((B * S, D), dtype=np.float32)
    for p, (a, b) in enumerate(pairs):
        r0, r1 = res.results[2 * p], res.results[2 * p + 1]
        ya = (r0["yta"].astype(np.float32) + r1["yta"].astype(np.float32))
        yb = (r0["ytb"].astype(np.float32) + r1["ytb"].astype(np.float32))
        ya = ya.transpose(1, 0, 2).reshape(D, CA).T + b2[a]   # [CA, D]
        yb = yb[:, :, :CB].transpose(1, 0, 2).reshape(D, CB).T + b2[b]
        out[tok[a]] = ya[: len(tok[a])]
        out[tok[b]] = yb[: len(tok[b])]
    for i, idx in overflow:
        hh = np.maximum(xf[idx] @ W1[i].T + b1[i], 0.0)
        out[idx] = hh @ W2[i].T + b2[i]
    return out.reshape(B, S, D)
